# revision 21
# baseline (speedup 1.0000x reference)
"""AM-softmax + hard-negative-mining loss (partial-FC style) on 8 TRN2 cores.

Strategy (classification/tensor parallel over the queue dim Q):
  - Column dedup: the blended weight w = mask*q1 + (1-mask)*q0 equals q0
    exactly where mask == 0 (~90% of columns), so the host permutes
    columns into a shared "U" block (one matmul feeding both loss terms)
    and an "M" block (both variants computed). ~45% FLOP reduction.
  - fp8(e4m3) matmuls in DoubleRow perf mode: inputs pre-scaled by 16 on
    host and quantized; each matmul contracts K=256 (two fp8 rows per PE
    cell); psum = 256*cos in fp32. fp8 end-to-end loss error ~6e-5 rel.
  - Batch rows reordered pos-first / outlier-last so each 128-row chunk
    needs only one kind of consumer: exp+rowsum (pos chunks, feeding
    logsumexp) or top-8-per-span (outlier chunks, feeding hard-negative
    top-k). That halves elementwise work vs exp+max8 everywhere.
  - A single [128, 4096] PSUM tile is used as a ring (the tile framework
    tracks subtile dependencies) with engine-exclusive regions: two
    1536-wide regions ping-pong the ACT stream (exact exp(32cos) with
    fused row-sum accumulation, in-place psum->psum, zero-gap); two
    512-wide regions feed the DVE stream.
  - DVE work: per pos chunk, U[0:1536] uses a Schraudolph bit-trick exp
    (y = int32(A*psum + B) reinterpreted as f32; the sawtooth error
    averages out inside the 65536-term logsumexp, validated ~5e-5 rel)
    followed by a reduce over the bitcast values; outlier chunks run
    max8 straight from PSUM in 512-wide spans. The two streams are
    merged by estimated engine time so ACT (~51us) and DVE (~46us) run
    concurrently; fills (PE, ~28us) and DMA (~15us) hide underneath.
  - Latency trims: per-bc p-slice DMAs + a split 512-wide first unit
    start ACT at ~4.5us; a dummy-matmul warmup starts the PE p-state
    ramp clock at ~1us (cost-model matmuls dispatched 3us after the
    ramp start run at full clock); a dummy activation pre-loads the Exp
    table during the DMA window; one merged output DMA at the end.
  - Cross-core/term merge (logsumexp adjust at the ground-truth column,
    top-k merge, masked means) happens on host in float64.
"""
import sys

sys.path.insert(0, "/opt/trn_rl_repo")

import numpy as np
import ml_dtypes

B = 1024
Q = 65536
D = 512
MARGIN = 0.4
SCALE = 32.0
HARD_NEG = 10
NCORES = 8
BC = B // 128              # 8 batch chunks

NU = 7424                  # U (shared) columns per core; capacity 59392
NM = 832                   # M (masked) columns per core; capacity 6656
# Column chunks: the D chunk (U[0:1536]) is consumed via DVE approx-exp
# for pos chunks; A chunks via ACT exact exp. M0/M1 are the two masked
# variants. For outlier (neg) chunks everything is consumed by DVE max8
# in 512-wide spans.
D_W = 1792                 # U columns offloaded to DVE per pos chunk
A_CHUNKS = [(1792, 1280), (3072, 1536), (4608, 1536), (6144, 1280)]
NSP_S = 11                 # sum spans: d0..d3 a0..a3 m0 m1 a0x
NSP_C = 20                 # cand spans (512-wide per neg chunk)
FSCALE = 16.0              # host pre-scale on p and q before fp8 quant
PSCALE = FSCALE * FSCALE   # psum = PSCALE * cos
MMW = 512                  # output cols per DoubleRow matmul
RING = 4096                # psum ring size (fp32 elements; 8 banks)

# Schraudolph approx exp on psum values x = PSCALE*cos:
#   exp(SCALE*cos) ~ bitcast_f32(int32(EXPA * x + EXPB))
EXPA = (2.0 ** 23) * 1.4426950408889634 * (SCALE / PSCALE)
EXPB = float((127 << 23) - 486411)

QS = Q // NCORES           # generic-fallback shard size
PW = 1024                  # generic fallback tile width
NSP_G = QS // PW

TRACE = False
LAST = {}

_NC_CACHE = {}


def _build_fast(kinds):
    """kinds: per-bc tuple of (needs_sum, needs_cand)."""
    key = ("fast", kinds)
    if key in _NC_CACHE:
        return _NC_CACHE[key]
    import concourse.mybir as mybir
    import concourse.tile as tile
    from concourse import bacc

    dt = mybir.dt
    f8 = dt.float8e4
    EXP = mybir.ActivationFunctionType.Exp
    DR = mybir.MatmulPerfMode.DoubleRow
    AX = mybir.AxisListType.X
    ALU = mybir.AluOpType
    nc = bacc.Bacc(None)

    pQ = nc.dram_tensor("pQ", [128, 2, 2, B], dt.uint8, kind="ExternalInput")
    qU = nc.dram_tensor("qU", [128, 2, 2, NU], dt.uint8, kind="ExternalInput")
    qM = nc.dram_tensor("qM", [128, 2, 2, 2 * NM], dt.uint8,
                        kind="ExternalInput")
    n_sum = sum(1 for s, _ in kinds if s)
    n_cand = sum(1 for _, c in kinds if c)
    n_out = n_sum * NSP_S + n_cand * NSP_C * 8
    out = nc.dram_tensor("out", [128, n_out], dt.float32,
                         kind="ExternalOutput")

    pos_bcs = [bc for bc in range(BC) if kinds[bc][0]]
    neg_bcs = [bc for bc in range(BC) if not kinds[bc][0]]
    si_row = {bc: i for i, bc in enumerate(bc for bc in range(BC)
                                           if kinds[bc][0])}
    ci_row = {bc: i for i, bc in enumerate(bc for bc in range(BC)
                                           if kinds[bc][1])}

    with tile.TileContext(nc) as tc:
        with (
            tc.tile_pool(name="const", bufs=1) as cpool,
            tc.tile_pool(name="scr", bufs=3) as spool,
            tc.tile_pool(name="ps", bufs=1, space="PSUM") as ps,
        ):
            ring = ps.tile([128, RING], dt.float32, name="ring")

            # -- warmups: start PE ramp clock + load ACT Exp table early
            wt = cpool.tile([128, 16], f8, name="wt")
            nc.vector.memset(wt[:], 0.0)
            wa = cpool.tile([128, 8], dt.float32, name="wa")
            nc.vector.memset(wa[:], 0.0)
            for i in range(12):
                nc.tensor.matmul(ring[0:1, 0:8], wt[:, 0:1], wt[:, 8:16],
                                 start=True, stop=True)
            nc.scalar.activation(wa[:], wa[:], EXP, scale=1.0)

            # DMA order = consumption order. The very first compute unit
            # is a 512-wide exp for bc0, so ship bc0's p slice and the
            # first 512 queue columns first to start ACT ~4us earlier.
            bc_order = pos_bcs + neg_bcs
            pQt = cpool.tile([128, 2, 2, B], f8, name="pQt")
            qUt = cpool.tile([128, 2, 2, NU], f8, name="qUt")

            def dma_pq(bc):
                b0 = bc * 128
                nc.sync.dma_start(pQt[:, :, :, b0:b0 + 128],
                                  pQ[:, :, :, b0:b0 + 128].bitcast(f8))

            # interleave per-bc p slices with the first queue chunks so the
            # k-th consumer unit's inputs arrive as early as possible
            dma_pq(bc_order[0])
            nc.sync.dma_start(qUt[:, :, :, 1792:2304],
                              qU[:, :, :, 1792:2304].bitcast(f8))
            dma_pq(bc_order[1])
            nc.sync.dma_start(qUt[:, :, :, 2304:3072],
                              qU[:, :, :, 2304:3072].bitcast(f8))
            for bc in bc_order[2:5]:
                dma_pq(bc)
            nc.sync.dma_start(qUt[:, :, :, 0:1792],
                              qU[:, :, :, 0:1792].bitcast(f8))
            for bc in bc_order[5:]:
                dma_pq(bc)
            for c0, w in A_CHUNKS[1:]:
                nc.sync.dma_start(qUt[:, :, :, c0:c0 + w],
                                  qU[:, :, :, c0:c0 + w].bitcast(f8))
            qMt = cpool.tile([128, 2, 2, 2 * NM], f8, name="qMt")
            nc.sync.dma_start(qMt[:], qM[:].bitcast(f8))

            outs = cpool.tile([128, max(n_out, 1)], dt.float32, name="outs")
            sums = outs[:, :n_sum * NSP_S].rearrange(
                "p (r s) -> p r s", s=NSP_S) if n_sum else outs
            cands = outs[:, n_sum * NSP_S:].rearrange(
                "p (r s) -> p r s", s=NSP_C * 8) if n_cand else outs
            nc.vector.memset(outs[:, :max(n_sum, 1) * NSP_S], 0.0)

            # Two independent unit streams on disjoint ring regions:
            #   ACT stream: R0 [0:1536] / R1 [1536:3072], alternating
            #   DVE stream: R2a [3072:3584] / R2b [3584:4096], alternating
            # ACT stream: (bc, src, c0, w, span) exact exp + accum, pos bcs
            act_stream = []
            for k, (c0, w) in enumerate(A_CHUNKS):
                for j, bc in enumerate(pos_bcs):
                    if k == 0 and j == 0:
                        # split bc0's first unit so ACT starts right after
                        # the small lead DMAs
                        act_stream.append((bc, qUt, c0, 512, 10))
                        act_stream.append((bc, qUt, c0 + 512, w - 512, 4))
                    else:
                        act_stream.append((bc, qUt, c0, w, 4 + k))
            for bc in pos_bcs:
                act_stream.append((bc, qMt, 0, NM, 8))
            for bc in pos_bcs:
                act_stream.append((bc, qMt, NM, NM, 9))

            # DVE stream: approx-exp D spans (pos) + max8 spans (neg),
            # all 512-wide. (kind, bc, src, c0, w, span)
            dve_stream = []
            d_units = []
            h0 = 0
            while h0 < D_W:
                d_units.append((h0, min(512, D_W - h0)))
                h0 += min(512, D_W - h0)
            for j, (c0, w) in enumerate(d_units):
                for bc in pos_bcs:
                    dve_stream.append(("exp", bc, qUt, c0, w, j))
            cs = 0
            for c0, w in [(0, D_W)] + A_CHUNKS:
                for h0 in range(0, w, 512):
                    hw = min(512, w - h0)
                    for bc in neg_bcs:
                        dve_stream.append(
                            ("max", bc, qUt, c0 + h0, hw, cs))
                    cs += 1
            for mv in range(2):
                for h0 in range(0, NM, 512):
                    hw = min(512, NM - h0)
                    for bc in neg_bcs:
                        dve_stream.append(
                            ("max", bc, qMt, mv * NM + h0, hw, cs))
                    cs += 1
            assert cs == NSP_C

            # merge the two streams by estimated engine time so both
            # engines stay fed from the shared fill producer
            def a_cost(u):
                return u[3] * 0.83 + 330.0

            def d_cost(u):
                return u[4] * (2.1 if u[0] == "exp" else 1.05) + 150.0

            plan = []
            ta = td = 0.0
            ia = idv = 0
            # seed: the first A-units go first (their q chunk is DMA'd
            # first and PE fills are in program order)
            seed = min(4, len(act_stream))
            for u in act_stream[:seed]:
                plan.append(("A",) + u)
                ta += a_cost(u)
            ia = seed
            td = ta * 0.45
            while ia < len(act_stream) or idv < len(dve_stream):
                if idv >= len(dve_stream) or (
                        ia < len(act_stream) and
                        ta + a_cost(act_stream[ia]) <=
                        td + d_cost(dve_stream[idv])):
                    plan.append(("A",) + act_stream[ia])
                    ta += a_cost(act_stream[ia])
                    ia += 1
                else:
                    plan.append(("D",) + dve_stream[idv])
                    td += d_cost(dve_stream[idv])
                    idv += 1

            flip = {"A": 0, "D": 0}
            base_for = {("A", 0): 0, ("A", 1): 1536,
                        ("D", 0): 3072, ("D", 1): 3584}

            for item in plan:
                if item[0] == "A":
                    _, bc, src, c0, w, sj = item
                    dkind = None
                else:
                    _, dkind, bc, src, c0, w, sj = item
                eng = item[0]
                flip[eng] ^= 1
                base = base_for[(eng, flip[eng])]
                acc = ring[:, base:base + w]
                for h0 in range(0, w, MMW):
                    hw = min(MMW, w - h0)
                    for dc in range(2):
                        nc.tensor.matmul(
                            acc[:, h0:h0 + hw],
                            pQt[:, dc, :, bc * 128:(bc + 1) * 128],
                            src[:, dc, :, c0 + h0:c0 + h0 + hw],
                            start=(dc == 0),
                            stop=(dc == 1),
                            perf_mode=DR,
                        )
                if eng == "A":
                    nc.scalar.activation(
                        acc, acc, EXP, scale=SCALE / PSCALE,
                        accum_out=sums[:, si_row[bc], sj:sj + 1],
                    )
                elif dkind == "exp":
                    # schraudolph approx exp to int32 (frees the ring
                    # region), then reduce over the bitcast floats
                    it = spool.tile([128, 512], dt.int32, tag="it",
                                    name=f"i{bc}_{sj}")
                    nc.vector.tensor_scalar(
                        it[:, 0:w], acc, EXPA, EXPB, ALU.mult, ALU.add)
                    nc.vector.tensor_reduce(
                        sums[:, si_row[bc], sj:sj + 1],
                        it[:, 0:w].bitcast(dt.float32), AX, ALU.add)
                else:
                    nc.vector.max(
                        out=cands[:, ci_row[bc], sj * 8:(sj + 1) * 8],
                        in_=acc)

            nc.sync.dma_start(out[:], outs[:])

    nc.compile()
    _NC_CACHE[key] = nc
    return nc


def _q_layout(rows, n_cols):
    """[k, D] fp8-bytes (k <= n_cols) -> [128, 2, 2, n_cols] uint8, zero pad.
    Element (pp, dc, i, j) = rows[j, dc*256 + i*128 + pp]."""
    out = np.zeros((128, 2, 2, n_cols), dtype=np.uint8)
    k = rows.shape[0]
    if k:
        t = np.ascontiguousarray(rows.T).reshape(2, 2, 128, k)
        out[:, :, :, :k] = t.transpose(2, 0, 1, 3)
    return np.ascontiguousarray(out)


def _fp8(x):
    return (np.asarray(x, np.float32) * FSCALE).astype(
        ml_dtypes.float8_e4m3).view(np.uint8)


# ---------------------------------------------------------------------------
# generic fallback (dense masks): f32r path, every column handled as masked
# ---------------------------------------------------------------------------

def _build_generic():
    if "gen" in _NC_CACHE:
        return _NC_CACHE["gen"]
    import concourse.mybir as mybir
    import concourse.tile as tile
    from concourse import bacc

    dt = mybir.dt
    nc = bacc.Bacc(None)
    f32r = dt.float32r
    EXP = mybir.ActivationFunctionType.Exp
    DCg = D // 128
    pT = nc.dram_tensor("pT", [DCg, 128, B], dt.float32, kind="ExternalInput")
    q0T = nc.dram_tensor("q0T", [128, DCg, QS], dt.float32, kind="ExternalInput")
    wT = nc.dram_tensor("wT", [128, DCg, QS], dt.float32, kind="ExternalInput")
    osums = nc.dram_tensor("osums", [2, BC, 128, NSP_G], dt.float32, kind="ExternalOutput")
    ocand = nc.dram_tensor("ocand", [2, BC, 128, NSP_G * 8], dt.float32, kind="ExternalOutput")

    with tile.TileContext(nc) as tc:
        with (
            tc.tile_pool(name="const", bufs=1) as cpool,
            tc.tile_pool(name="qin", bufs=4) as qpool,
            tc.tile_pool(name="accum", bufs=1) as apool,
            tc.tile_pool(name="scr", bufs=3) as spool,
            tc.tile_pool(name="ps", bufs=4, space="PSUM") as ps,
        ):
            pTr = cpool.tile([128, DCg, B], f32r, tag="pTr", name="pTr")
            for dcg in range(DCg):
                nc.sync.dma_start(pTr[:, dcg, :], pT[dcg].bitcast(f32r))

            sums = [[apool.tile([128, NSP_G], dt.float32, tag=f"s{m}_{bc}",
                                name=f"s{m}_{bc}") for bc in range(BC)]
                    for m in range(2)]
            cand = [[apool.tile([128, NSP_G * 8], dt.float32, tag=f"c{m}_{bc}",
                                name=f"c{m}_{bc}") for bc in range(BC)]
                    for m in range(2)]

            for m, src_dram in enumerate((q0T, wT)):
                for si in range(NSP_G):
                    off = si * PW
                    qt = qpool.tile([128, DCg, PW], f32r, tag="q",
                                    name=f"g{m}q{si}")
                    for dcg in range(DCg):
                        nc.sync.dma_start(
                            qt[:, dcg, :],
                            src_dram[:, dcg, off:off + PW].bitcast(f32r))
                    for bc in range(BC):
                        acc = ps.tile([128, PW], dt.float32, tag="ps",
                                      name=f"g{m}a{si}_{bc}")
                        for h0 in range(0, PW, 512):
                            for dcg in range(DCg):
                                nc.tensor.matmul(
                                    acc[:, h0:h0 + 512],
                                    pTr[:, dcg, bc * 128:(bc + 1) * 128],
                                    qt[:, dcg, h0:h0 + 512],
                                    start=(dcg == 0),
                                    stop=(dcg == DCg - 1),
                                )
                        et = spool.tile([128, PW], dt.float32, tag="et",
                                        name=f"g{m}e{si}_{bc}")
                        nc.scalar.activation(
                            et[:], acc[:], EXP, scale=SCALE,
                            accum_out=sums[m][bc][:, si:si + 1],
                        )
                        nc.vector.max(
                            out=cand[m][bc][:, si * 8:(si + 1) * 8],
                            in_=et[:])

            for m in range(2):
                for bc in range(BC):
                    nc.sync.dma_start(osums[m, bc], sums[m][bc][:])
                    nc.sync.dma_start(ocand[m, bc], cand[m][bc][:])

    nc.compile()
    _NC_CACHE["gen"] = nc
    return nc


def _layoutT(cols_2d, n_cols):
    DCg = D // 128
    out = np.zeros((128, DCg, n_cols), dtype=np.float32)
    k = cols_2d.shape[0]
    if k:
        t = np.ascontiguousarray(cols_2d.T).reshape(DCg, 128, k)
        out[:, :, :k] = t.transpose(1, 0, 2)
    return np.ascontiguousarray(out)


def _host_loss(p, queue, mask_flat, label, z_sums, cand_cos):
    """z_sums: [2, B] fp64 raw exp-sums (pads already removed);
    cand_cos: [2][B, ncand] fp64 candidate cos values."""
    pos_mask = label != -1
    n_pos = int(pos_mask.sum())
    n_neg = B - n_pos
    p64 = p.astype(np.float64)
    q64 = queue.astype(np.float64)
    m64 = mask_flat.astype(np.float64)

    loss = 0.0
    for m in range(2):
        if n_pos > 0:
            lbl = label[pos_mask]
            if m == 0:
                w_rows = q64[0, lbl, :]
            else:
                mm = m64[lbl][:, None]
                w_rows = mm * q64[1, lbl, :] + (1.0 - mm) * q64[0, lbl, :]
            gt = np.einsum("bd,bd->b", p64[pos_mask], w_rows)
            z = z_sums[m][pos_mask]
            z_adj = z - np.exp(SCALE * gt) + np.exp(SCALE * (gt - MARGIN))
            ce = np.log(z_adj) - (gt - MARGIN) * SCALE
            loss += ce.sum() / max(n_pos, 1)
        if n_neg > 0:
            co = cand_cos[m][~pos_mask]
            topk = -np.partition(-co, HARD_NEG - 1, axis=1)[:, :HARD_NEG]
            hard = np.clip(topk, 0.0, None)
            loss += hard.mean(axis=1).sum() / max(n_neg, 1)
    return np.float32(loss)


def kernel(p, queue, mask, label):
    from concourse.bass_utils import run_bass_kernel_spmd

    p = np.ascontiguousarray(np.asarray(p, dtype=np.float32))
    queue = np.asarray(queue, dtype=np.float32)
    mask_flat = np.asarray(mask, dtype=np.float32).reshape(-1)
    label = np.asarray(label).astype(np.int64).reshape(-1)

    mask_nz = mask_flat != 0.0
    idx_M = np.nonzero(mask_nz)[0]
    idx_U = np.nonzero(~mask_nz)[0]
    use_fast = len(idx_M) <= NCORES * NM and len(idx_U) <= NCORES * NU

    core_ids = list(range(NCORES))
    kw = {}
    if TRACE:
        kw = dict(trace=True, trace_cores=[0])

    if not use_fast:
        # dense/sparse-extreme masks: f32r generic path (2 matmuls/col)
        perm = np.concatenate([idx_U, idx_M])
        q0p = queue[0, perm, :]
        mcol = mask_flat[perm][:, None]
        wp = (mcol * queue[1, perm, :] + (1.0 - mcol) * queue[0, perm, :]
              ).astype(np.float32)
        pT = np.ascontiguousarray(p.T).reshape(D // 128, 128, B)
        in_maps = []
        for c in core_ids:
            sl = slice(c * QS, (c + 1) * QS)
            in_maps.append({
                "pT": pT,
                "q0T": _layoutT(q0p[sl], QS),
                "wT": _layoutT(wp[sl], QS),
            })
        nc = _build_generic()
        try:
            res = run_bass_kernel_spmd(nc, in_maps, core_ids, **kw)
        except ModuleNotFoundError:
            res = run_bass_kernel_spmd(nc, in_maps, core_ids)
        LAST["res"] = res
        z_sums = np.zeros((2, B), dtype=np.float64)
        cands = [[], []]
        for c in core_ids:
            r = res.results[c]
            z_sums += r["osums"].astype(np.float64).sum(axis=3).reshape(2, B)
            cm = r["ocand"].astype(np.float64).reshape(2, B, NSP_G * 8)
            cands[0].append(cm[0])
            cands[1].append(cm[1])
        with np.errstate(divide="ignore"):
            cand_cos = [np.log(np.concatenate(cands[0], axis=1)) / SCALE,
                        np.log(np.concatenate(cands[1], axis=1)) / SCALE]
        return _host_loss(p, queue, mask_flat, label, z_sums, cand_cos)

    # ---- fast path ----
    pos_mask_orig = label != -1
    perm_rows = np.argsort(~pos_mask_orig, kind="stable")
    p_r = p[perm_rows]
    pos_r = pos_mask_orig[perm_rows]
    kinds = tuple(
        (bool(pos_r[bc * 128:(bc + 1) * 128].any()),
         bool((~pos_r[bc * 128:(bc + 1) * 128]).any()))
        for bc in range(BC))

    q0 = queue[0]
    mcolM = mask_flat[idx_M][:, None]
    wM = (mcolM * queue[1, idx_M, :]
          + (1.0 - mcolM) * queue[0, idx_M, :]).astype(np.float32)

    p8 = _fp8(p_r)                     # [B, D] u8
    pQ = np.ascontiguousarray(
        p8.T.reshape(2, 2, 128, B).transpose(2, 0, 1, 3))
    q0_8 = _fp8(q0)
    wM_8 = _fp8(wM)

    in_maps = []
    pads = []
    for c in core_ids:
        iu = idx_U[c * NU:(c + 1) * NU]
        im = slice(c * NM, min((c + 1) * NM, len(idx_M)))
        m_rows = wM_8[im]
        m0_rows = q0_8[idx_M[im]]
        qm = np.zeros((128, 2, 2, 2 * NM), dtype=np.uint8)
        qm[:, :, :, :NM] = _q_layout(m0_rows, NM)
        qm[:, :, :, NM:] = _q_layout(m_rows, NM)
        in_maps.append({
            "pQ": pQ,
            "qU": _q_layout(q0_8[iu], NU),
            "qM": qm,
        })
        pads.append((NU - len(iu)) + (NM - m0_rows.shape[0]))

    nc = _build_fast(kinds)
    try:
        res = run_bass_kernel_spmd(nc, in_maps, core_ids, **kw)
    except ModuleNotFoundError:
        res = run_bass_kernel_spmd(nc, in_maps, core_ids)
    LAST["res"] = res

    # ---- host-side reduction (float64) ----
    n_sum = sum(1 for s, _ in kinds if s)
    n_cand = sum(1 for _, c in kinds if c)
    sum_rows = [bc for bc in range(BC) if kinds[bc][0]]
    cand_rows = [bc for bc in range(BC) if kinds[bc][1]]

    z_r = np.zeros((2, B), dtype=np.float64)
    cand_chunks = [[[] for _ in range(BC)] for _ in range(2)]
    pad_tot = 0.0
    for c in core_ids:
        r = res.results[c]
        ro = r["out"].astype(np.float64)
        su = ro[:, :n_sum * NSP_S].reshape(128, n_sum, NSP_S)
        for k_i, bc in enumerate(sum_rows):
            rows = slice(bc * 128, (bc + 1) * 128)
            u_part = su[:, k_i, :8].sum(axis=1) + su[:, k_i, 10]
            z_r[0, rows] += u_part + su[:, k_i, 8]
            z_r[1, rows] += u_part + su[:, k_i, 9]
        cu = ro[:, n_sum * NSP_S:].reshape(128, n_cand, NSP_C, 8)
        for k_i, bc in enumerate(cand_rows):
            for m in range(2):
                sel = list(range(16)) + [16 + 2 * m, 17 + 2 * m]
                cand_chunks[m][bc].append(
                    cu[:, k_i, sel, :].reshape(128, -1) / PSCALE)
        pad_tot += pads[c]
    z_r -= pad_tot

    z_sums = np.zeros((2, B), dtype=np.float64)
    z_sums[:, perm_rows] = z_r

    ncc = 18 * 8 * NCORES
    cand_cos = []
    for m in range(2):
        cc = np.full((B, ncc), -1.0)
        for bc in cand_rows:
            rows = slice(bc * 128, (bc + 1) * 128)
            cc[rows] = np.concatenate(cand_chunks[m][bc], axis=1)
        cc_orig = np.full_like(cc, -1.0)
        cc_orig[perm_rows] = cc
        cand_cos.append(cc_orig)

    return _host_loss(p, queue, mask_flat, label, z_sums, cand_cos)


# revision 22
# speedup vs baseline: 1.0764x; 1.0764x over previous
"""AM-softmax + hard-negative-mining loss (partial-FC style) on 8 TRN2 cores.

Strategy (classification/tensor parallel over the queue dim Q):
  - Column dedup: the blended weight w = mask*q1 + (1-mask)*q0 equals q0
    exactly where mask == 0 (~90% of columns), so the host permutes
    columns into a shared "U" block (one matmul feeding both loss terms)
    and an "M" block (both variants computed). ~45% FLOP reduction.
  - fp8(e4m3) matmuls in DoubleRow perf mode: inputs pre-scaled by 16 on
    host and quantized; each matmul contracts K=256 (two fp8 rows per PE
    cell); psum = 256*cos in fp32. fp8 end-to-end loss error ~6e-5 rel.
  - Batch rows reordered pos-first / outlier-last so each 128-row chunk
    needs only one kind of consumer: exp+rowsum (pos chunks, feeding
    logsumexp) or top-8-per-span (outlier chunks, feeding hard-negative
    top-k). That halves elementwise work vs exp+max8 everywhere.
  - A single [128, 4096] PSUM tile is used as a ring (the tile framework
    tracks subtile dependencies) with engine-exclusive regions: two
    1536-wide regions ping-pong the ACT stream (exact exp(32cos) with
    fused row-sum accumulation, in-place psum->psum, zero-gap); two
    512-wide regions feed the DVE stream.
  - DVE work: per pos chunk, U[0:1536] uses a Schraudolph bit-trick exp
    (y = int32(A*psum + B) reinterpreted as f32; the sawtooth error
    averages out inside the 65536-term logsumexp, validated ~5e-5 rel)
    followed by a reduce over the bitcast values; outlier chunks run
    max8 straight from PSUM in 512-wide spans. The two streams are
    merged by estimated engine time so ACT (~51us) and DVE (~46us) run
    concurrently; fills (PE, ~28us) and DMA (~15us) hide underneath.
  - Latency trims: per-bc p-slice DMAs + a split 512-wide first unit
    start ACT at ~4.5us; a dummy-matmul warmup starts the PE p-state
    ramp clock at ~1us (cost-model matmuls dispatched 3us after the
    ramp start run at full clock); a dummy activation pre-loads the Exp
    table during the DMA window; one merged output DMA at the end.
  - Cross-core/term merge (logsumexp adjust at the ground-truth column,
    top-k merge, masked means) happens on host in float64.
"""
import sys

sys.path.insert(0, "/opt/trn_rl_repo")

import numpy as np
import ml_dtypes

B = 1024
Q = 65536
D = 512
MARGIN = 0.4
SCALE = 32.0
HARD_NEG = 10
NCORES = 8
BC = B // 128              # 8 batch chunks

NU = 7424                  # U (shared) columns per core; capacity 59392
NM = 832                   # M (masked) columns per core; capacity 6656
# Column chunks: the D chunk (U[0:1536]) is consumed via DVE approx-exp
# for pos chunks; A chunks via ACT exact exp. M0/M1 are the two masked
# variants. For outlier (neg) chunks everything is consumed by DVE max8
# in 512-wide spans.
D_W = 1536                 # U columns offloaded to DVE per pos chunk
A_CHUNKS = [(1536, 1536), (3072, 1536), (4608, 1536), (6144, 1280)]
NSP_S = 10                 # sum spans: d0 d1 d2 a0 a1 a2 a3 m0 m1 a0x
NSP_C = 19                 # cand spans (512-wide per neg chunk)
FSCALE = 16.0              # host pre-scale on p and q before fp8 quant
PSCALE = FSCALE * FSCALE   # psum = PSCALE * cos
MMW = 512                  # output cols per DoubleRow matmul
RING = 4096                # psum ring size (fp32 elements; 8 banks)

# Schraudolph approx exp on psum values x = PSCALE*cos:
#   exp(SCALE*cos) ~ bitcast_f32(int32(EXPA * x + EXPB))
EXPA = (2.0 ** 23) * 1.4426950408889634 * (SCALE / PSCALE)
EXPB = float((127 << 23) - 486411)

QS = Q // NCORES           # generic-fallback shard size
PW = 1024                  # generic fallback tile width
NSP_G = QS // PW

TRACE = False
LAST = {}

_NC_CACHE = {}


def _build_fast(kinds):
    """kinds: per-bc tuple of (needs_sum, needs_cand)."""
    key = ("fast", kinds)
    if key in _NC_CACHE:
        return _NC_CACHE[key]
    import concourse.mybir as mybir
    import concourse.tile as tile
    from concourse import bacc

    dt = mybir.dt
    f8 = dt.float8e4
    EXP = mybir.ActivationFunctionType.Exp
    DR = mybir.MatmulPerfMode.DoubleRow
    AX = mybir.AxisListType.X
    ALU = mybir.AluOpType
    nc = bacc.Bacc(None)

    pQ = nc.dram_tensor("pQ", [128, 2, 2, B], dt.uint8, kind="ExternalInput")
    qU = nc.dram_tensor("qU", [128, 2, 2, NU], dt.uint8, kind="ExternalInput")
    qM = nc.dram_tensor("qM", [128, 2, 2, 2 * NM], dt.uint8,
                        kind="ExternalInput")
    n_sum = sum(1 for s, _ in kinds if s)
    n_cand = sum(1 for _, c in kinds if c)
    n_out = n_sum * NSP_S + n_cand * NSP_C * 8
    out = nc.dram_tensor("out", [128, n_out], dt.float32,
                         kind="ExternalOutput")

    pos_bcs = [bc for bc in range(BC) if kinds[bc][0]]
    neg_bcs = [bc for bc in range(BC) if not kinds[bc][0]]
    si_row = {bc: i for i, bc in enumerate(bc for bc in range(BC)
                                           if kinds[bc][0])}
    ci_row = {bc: i for i, bc in enumerate(bc for bc in range(BC)
                                           if kinds[bc][1])}

    with tile.TileContext(nc) as tc:
        with (
            tc.tile_pool(name="const", bufs=1) as cpool,
            tc.tile_pool(name="scr", bufs=3) as spool,
            tc.tile_pool(name="ps", bufs=1, space="PSUM") as ps,
        ):
            ring = ps.tile([128, RING], dt.float32, name="ring")

            # -- warmups: start PE ramp clock + load ACT Exp table early
            wt = cpool.tile([128, 16], f8, name="wt")
            nc.vector.memset(wt[:], 0.0)
            wa = cpool.tile([128, 8], dt.float32, name="wa")
            nc.vector.memset(wa[:], 0.0)
            for i in range(12):
                nc.tensor.matmul(ring[0:1, 0:8], wt[:, 0:1], wt[:, 8:16],
                                 start=True, stop=True)
            nc.scalar.activation(wa[:], wa[:], EXP, scale=1.0)

            # DMA order = consumption order. The very first compute unit
            # is a 512-wide exp for bc0, so ship bc0's p slice and the
            # first 512 queue columns first to start ACT ~4us earlier.
            bc_order = pos_bcs + neg_bcs
            pQt = cpool.tile([128, 2, 2, B], f8, name="pQt")
            qUt = cpool.tile([128, 2, 2, NU], f8, name="qUt")

            def dma_pq(bc):
                b0 = bc * 128
                nc.sync.dma_start(pQt[:, :, :, b0:b0 + 128],
                                  pQ[:, :, :, b0:b0 + 128].bitcast(f8))

            # interleave per-bc p slices with the first queue chunks so the
            # k-th consumer unit's inputs arrive as early as possible
            dma_pq(bc_order[0])
            nc.sync.dma_start(qUt[:, :, :, 1536:2048],
                              qU[:, :, :, 1536:2048].bitcast(f8))
            dma_pq(bc_order[1])
            nc.sync.dma_start(qUt[:, :, :, 2048:3072],
                              qU[:, :, :, 2048:3072].bitcast(f8))
            for bc in bc_order[2:5]:
                dma_pq(bc)
            nc.sync.dma_start(qUt[:, :, :, 0:1536],
                              qU[:, :, :, 0:1536].bitcast(f8))
            for bc in bc_order[5:]:
                dma_pq(bc)
            for c0, w in A_CHUNKS[1:]:
                nc.sync.dma_start(qUt[:, :, :, c0:c0 + w],
                                  qU[:, :, :, c0:c0 + w].bitcast(f8))
            qMt = cpool.tile([128, 2, 2, 2 * NM], f8, name="qMt")
            nc.sync.dma_start(qMt[:], qM[:].bitcast(f8))

            outs = cpool.tile([128, max(n_out, 1)], dt.float32, name="outs")
            sums = outs[:, :n_sum * NSP_S].rearrange(
                "p (r s) -> p r s", s=NSP_S) if n_sum else outs
            cands = outs[:, n_sum * NSP_S:].rearrange(
                "p (r s) -> p r s", s=NSP_C * 8) if n_cand else outs
            nc.vector.memset(outs[:, :max(n_sum, 1) * NSP_S], 0.0)

            # Two independent unit streams on disjoint ring regions:
            #   ACT stream: R0 [0:1536] / R1 [1536:3072], alternating
            #   DVE stream: R2a [3072:3584] / R2b [3584:4096], alternating
            # ACT stream: (bc, src, c0, w, span) exact exp + accum, pos bcs
            act_stream = []
            for k, (c0, w) in enumerate(A_CHUNKS):
                for j, bc in enumerate(pos_bcs):
                    if k == 0 and j == 0:
                        # split bc0's first unit so ACT starts right after
                        # the small lead DMAs
                        act_stream.append((bc, qUt, c0, 512, 9))
                        act_stream.append((bc, qUt, c0 + 512, w - 512, 3))
                    else:
                        act_stream.append((bc, qUt, c0, w, 3 + k))
            for bc in pos_bcs:
                act_stream.append((bc, qMt, 0, NM, 7))
            for bc in pos_bcs:
                act_stream.append((bc, qMt, NM, NM, 8))

            # DVE stream: approx-exp D spans (pos) + max8 spans (neg),
            # all 512-wide. (kind, bc, src, c0, w, span)
            dve_stream = []
            for j in range(D_W // 512):
                for bc in pos_bcs:
                    dve_stream.append(("exp", bc, qUt, j * 512, 512, j))
            cs = 0
            for c0, w in [(0, D_W)] + A_CHUNKS:
                for h0 in range(0, w, 512):
                    hw = min(512, w - h0)
                    for bc in neg_bcs:
                        dve_stream.append(
                            ("max", bc, qUt, c0 + h0, hw, cs))
                    cs += 1
            for mv in range(2):
                for h0 in range(0, NM, 512):
                    hw = min(512, NM - h0)
                    for bc in neg_bcs:
                        dve_stream.append(
                            ("max", bc, qMt, mv * NM + h0, hw, cs))
                    cs += 1
            assert cs == NSP_C

            # merge the two streams by estimated engine time so both
            # engines stay fed from the shared fill producer
            def a_cost(u):
                return u[3] * 0.83 + 330.0

            def d_cost(u):
                return u[4] * (2.1 if u[0] == "exp" else 1.05) + 150.0

            plan = []
            ta = td = 0.0
            ia = idv = 0
            # seed: the first A-units go first (their q chunk is DMA'd
            # first and PE fills are in program order)
            seed = min(4, len(act_stream))
            for u in act_stream[:seed]:
                plan.append(("A",) + u)
                ta += a_cost(u)
            ia = seed
            td = ta * 0.45
            while ia < len(act_stream) or idv < len(dve_stream):
                if idv >= len(dve_stream) or (
                        ia < len(act_stream) and
                        ta + a_cost(act_stream[ia]) <=
                        td + d_cost(dve_stream[idv])):
                    plan.append(("A",) + act_stream[ia])
                    ta += a_cost(act_stream[ia])
                    ia += 1
                else:
                    plan.append(("D",) + dve_stream[idv])
                    td += d_cost(dve_stream[idv])
                    idv += 1

            flip = {"A": 0, "D": 0}
            base_for = {("A", 0): 0, ("A", 1): 1536,
                        ("D", 0): 3072, ("D", 1): 3584}

            for item in plan:
                if item[0] == "A":
                    _, bc, src, c0, w, sj = item
                    dkind = None
                else:
                    _, dkind, bc, src, c0, w, sj = item
                eng = item[0]
                flip[eng] ^= 1
                base = base_for[(eng, flip[eng])]
                acc = ring[:, base:base + w]
                for h0 in range(0, w, MMW):
                    hw = min(MMW, w - h0)
                    for dc in range(2):
                        nc.tensor.matmul(
                            acc[:, h0:h0 + hw],
                            pQt[:, dc, :, bc * 128:(bc + 1) * 128],
                            src[:, dc, :, c0 + h0:c0 + h0 + hw],
                            start=(dc == 0),
                            stop=(dc == 1),
                            perf_mode=DR,
                        )
                if eng == "A":
                    nc.scalar.activation(
                        acc, acc, EXP, scale=SCALE / PSCALE,
                        accum_out=sums[:, si_row[bc], sj:sj + 1],
                    )
                elif dkind == "exp":
                    # schraudolph approx exp to int32 (frees the ring
                    # region), then reduce over the bitcast floats
                    it = spool.tile([128, 512], dt.int32, tag="it",
                                    name=f"i{bc}_{sj}")
                    nc.vector.tensor_scalar(
                        it[:, 0:w], acc, EXPA, EXPB, ALU.mult, ALU.add)
                    nc.vector.tensor_reduce(
                        sums[:, si_row[bc], sj:sj + 1],
                        it[:, 0:w].bitcast(dt.float32), AX, ALU.add)
                else:
                    nc.vector.max(
                        out=cands[:, ci_row[bc], sj * 8:(sj + 1) * 8],
                        in_=acc)

            nc.sync.dma_start(out[:], outs[:])

    nc.compile()
    _NC_CACHE[key] = nc
    return nc


def _q_layout(rows, n_cols):
    """[k, D] fp8-bytes (k <= n_cols) -> [128, 2, 2, n_cols] uint8, zero pad.
    Element (pp, dc, i, j) = rows[j, dc*256 + i*128 + pp]."""
    out = np.zeros((128, 2, 2, n_cols), dtype=np.uint8)
    k = rows.shape[0]
    if k:
        t = np.ascontiguousarray(rows.T).reshape(2, 2, 128, k)
        out[:, :, :, :k] = t.transpose(2, 0, 1, 3)
    return np.ascontiguousarray(out)


def _fp8(x):
    return (np.asarray(x, np.float32) * FSCALE).astype(
        ml_dtypes.float8_e4m3).view(np.uint8)


# ---------------------------------------------------------------------------
# generic fallback (dense masks): f32r path, every column handled as masked
# ---------------------------------------------------------------------------

def _build_generic():
    if "gen" in _NC_CACHE:
        return _NC_CACHE["gen"]
    import concourse.mybir as mybir
    import concourse.tile as tile
    from concourse import bacc

    dt = mybir.dt
    nc = bacc.Bacc(None)
    f32r = dt.float32r
    EXP = mybir.ActivationFunctionType.Exp
    DCg = D // 128
    pT = nc.dram_tensor("pT", [DCg, 128, B], dt.float32, kind="ExternalInput")
    q0T = nc.dram_tensor("q0T", [128, DCg, QS], dt.float32, kind="ExternalInput")
    wT = nc.dram_tensor("wT", [128, DCg, QS], dt.float32, kind="ExternalInput")
    osums = nc.dram_tensor("osums", [2, BC, 128, NSP_G], dt.float32, kind="ExternalOutput")
    ocand = nc.dram_tensor("ocand", [2, BC, 128, NSP_G * 8], dt.float32, kind="ExternalOutput")

    with tile.TileContext(nc) as tc:
        with (
            tc.tile_pool(name="const", bufs=1) as cpool,
            tc.tile_pool(name="qin", bufs=4) as qpool,
            tc.tile_pool(name="accum", bufs=1) as apool,
            tc.tile_pool(name="scr", bufs=3) as spool,
            tc.tile_pool(name="ps", bufs=4, space="PSUM") as ps,
        ):
            pTr = cpool.tile([128, DCg, B], f32r, tag="pTr", name="pTr")
            for dcg in range(DCg):
                nc.sync.dma_start(pTr[:, dcg, :], pT[dcg].bitcast(f32r))

            sums = [[apool.tile([128, NSP_G], dt.float32, tag=f"s{m}_{bc}",
                                name=f"s{m}_{bc}") for bc in range(BC)]
                    for m in range(2)]
            cand = [[apool.tile([128, NSP_G * 8], dt.float32, tag=f"c{m}_{bc}",
                                name=f"c{m}_{bc}") for bc in range(BC)]
                    for m in range(2)]

            for m, src_dram in enumerate((q0T, wT)):
                for si in range(NSP_G):
                    off = si * PW
                    qt = qpool.tile([128, DCg, PW], f32r, tag="q",
                                    name=f"g{m}q{si}")
                    for dcg in range(DCg):
                        nc.sync.dma_start(
                            qt[:, dcg, :],
                            src_dram[:, dcg, off:off + PW].bitcast(f32r))
                    for bc in range(BC):
                        acc = ps.tile([128, PW], dt.float32, tag="ps",
                                      name=f"g{m}a{si}_{bc}")
                        for h0 in range(0, PW, 512):
                            for dcg in range(DCg):
                                nc.tensor.matmul(
                                    acc[:, h0:h0 + 512],
                                    pTr[:, dcg, bc * 128:(bc + 1) * 128],
                                    qt[:, dcg, h0:h0 + 512],
                                    start=(dcg == 0),
                                    stop=(dcg == DCg - 1),
                                )
                        et = spool.tile([128, PW], dt.float32, tag="et",
                                        name=f"g{m}e{si}_{bc}")
                        nc.scalar.activation(
                            et[:], acc[:], EXP, scale=SCALE,
                            accum_out=sums[m][bc][:, si:si + 1],
                        )
                        nc.vector.max(
                            out=cand[m][bc][:, si * 8:(si + 1) * 8],
                            in_=et[:])

            for m in range(2):
                for bc in range(BC):
                    nc.sync.dma_start(osums[m, bc], sums[m][bc][:])
                    nc.sync.dma_start(ocand[m, bc], cand[m][bc][:])

    nc.compile()
    _NC_CACHE["gen"] = nc
    return nc


def _layoutT(cols_2d, n_cols):
    DCg = D // 128
    out = np.zeros((128, DCg, n_cols), dtype=np.float32)
    k = cols_2d.shape[0]
    if k:
        t = np.ascontiguousarray(cols_2d.T).reshape(DCg, 128, k)
        out[:, :, :k] = t.transpose(1, 0, 2)
    return np.ascontiguousarray(out)


def _host_loss(p, queue, mask_flat, label, z_sums, cand_cos):
    """z_sums: [2, B] fp64 raw exp-sums (pads already removed);
    cand_cos: [2][B, ncand] fp64 candidate cos values."""
    pos_mask = label != -1
    n_pos = int(pos_mask.sum())
    n_neg = B - n_pos
    p64 = p.astype(np.float64)
    q64 = queue.astype(np.float64)
    m64 = mask_flat.astype(np.float64)

    loss = 0.0
    for m in range(2):
        if n_pos > 0:
            lbl = label[pos_mask]
            if m == 0:
                w_rows = q64[0, lbl, :]
            else:
                mm = m64[lbl][:, None]
                w_rows = mm * q64[1, lbl, :] + (1.0 - mm) * q64[0, lbl, :]
            gt = np.einsum("bd,bd->b", p64[pos_mask], w_rows)
            z = z_sums[m][pos_mask]
            z_adj = z - np.exp(SCALE * gt) + np.exp(SCALE * (gt - MARGIN))
            ce = np.log(z_adj) - (gt - MARGIN) * SCALE
            loss += ce.sum() / max(n_pos, 1)
        if n_neg > 0:
            co = cand_cos[m][~pos_mask]
            topk = -np.partition(-co, HARD_NEG - 1, axis=1)[:, :HARD_NEG]
            hard = np.clip(topk, 0.0, None)
            loss += hard.mean(axis=1).sum() / max(n_neg, 1)
    return np.float32(loss)


def kernel(p, queue, mask, label):
    from concourse.bass_utils import run_bass_kernel_spmd

    p = np.ascontiguousarray(np.asarray(p, dtype=np.float32))
    queue = np.asarray(queue, dtype=np.float32)
    mask_flat = np.asarray(mask, dtype=np.float32).reshape(-1)
    label = np.asarray(label).astype(np.int64).reshape(-1)

    mask_nz = mask_flat != 0.0
    idx_M = np.nonzero(mask_nz)[0]
    idx_U = np.nonzero(~mask_nz)[0]
    use_fast = len(idx_M) <= NCORES * NM and len(idx_U) <= NCORES * NU

    core_ids = list(range(NCORES))
    kw = {}
    if TRACE:
        kw = dict(trace=True, trace_cores=[0])

    if not use_fast:
        # dense/sparse-extreme masks: f32r generic path (2 matmuls/col)
        perm = np.concatenate([idx_U, idx_M])
        q0p = queue[0, perm, :]
        mcol = mask_flat[perm][:, None]
        wp = (mcol * queue[1, perm, :] + (1.0 - mcol) * queue[0, perm, :]
              ).astype(np.float32)
        pT = np.ascontiguousarray(p.T).reshape(D // 128, 128, B)
        in_maps = []
        for c in core_ids:
            sl = slice(c * QS, (c + 1) * QS)
            in_maps.append({
                "pT": pT,
                "q0T": _layoutT(q0p[sl], QS),
                "wT": _layoutT(wp[sl], QS),
            })
        nc = _build_generic()
        try:
            res = run_bass_kernel_spmd(nc, in_maps, core_ids, **kw)
        except ModuleNotFoundError:
            res = run_bass_kernel_spmd(nc, in_maps, core_ids)
        LAST["res"] = res
        z_sums = np.zeros((2, B), dtype=np.float64)
        cands = [[], []]
        for c in core_ids:
            r = res.results[c]
            z_sums += r["osums"].astype(np.float64).sum(axis=3).reshape(2, B)
            cm = r["ocand"].astype(np.float64).reshape(2, B, NSP_G * 8)
            cands[0].append(cm[0])
            cands[1].append(cm[1])
        with np.errstate(divide="ignore"):
            cand_cos = [np.log(np.concatenate(cands[0], axis=1)) / SCALE,
                        np.log(np.concatenate(cands[1], axis=1)) / SCALE]
        return _host_loss(p, queue, mask_flat, label, z_sums, cand_cos)

    # ---- fast path ----
    pos_mask_orig = label != -1
    perm_rows = np.argsort(~pos_mask_orig, kind="stable")
    p_r = p[perm_rows]
    pos_r = pos_mask_orig[perm_rows]
    kinds = tuple(
        (bool(pos_r[bc * 128:(bc + 1) * 128].any()),
         bool((~pos_r[bc * 128:(bc + 1) * 128]).any()))
        for bc in range(BC))

    q0 = queue[0]
    mcolM = mask_flat[idx_M][:, None]
    wM = (mcolM * queue[1, idx_M, :]
          + (1.0 - mcolM) * queue[0, idx_M, :]).astype(np.float32)

    p8 = _fp8(p_r)                     # [B, D] u8
    pQ = np.ascontiguousarray(
        p8.T.reshape(2, 2, 128, B).transpose(2, 0, 1, 3))
    q0_8 = _fp8(q0)
    wM_8 = _fp8(wM)

    in_maps = []
    pads = []
    for c in core_ids:
        iu = idx_U[c * NU:(c + 1) * NU]
        im = slice(c * NM, min((c + 1) * NM, len(idx_M)))
        m_rows = wM_8[im]
        m0_rows = q0_8[idx_M[im]]
        qm = np.zeros((128, 2, 2, 2 * NM), dtype=np.uint8)
        qm[:, :, :, :NM] = _q_layout(m0_rows, NM)
        qm[:, :, :, NM:] = _q_layout(m_rows, NM)
        in_maps.append({
            "pQ": pQ,
            "qU": _q_layout(q0_8[iu], NU),
            "qM": qm,
        })
        pads.append((NU - len(iu)) + (NM - m0_rows.shape[0]))

    nc = _build_fast(kinds)
    try:
        res = run_bass_kernel_spmd(nc, in_maps, core_ids, **kw)
    except ModuleNotFoundError:
        res = run_bass_kernel_spmd(nc, in_maps, core_ids)
    LAST["res"] = res

    # ---- host-side reduction (float64) ----
    n_sum = sum(1 for s, _ in kinds if s)
    n_cand = sum(1 for _, c in kinds if c)
    sum_rows = [bc for bc in range(BC) if kinds[bc][0]]
    cand_rows = [bc for bc in range(BC) if kinds[bc][1]]

    z_r = np.zeros((2, B), dtype=np.float64)
    cand_chunks = [[[] for _ in range(BC)] for _ in range(2)]
    pad_tot = 0.0
    for c in core_ids:
        r = res.results[c]
        ro = r["out"].astype(np.float64)
        su = ro[:, :n_sum * NSP_S].reshape(128, n_sum, NSP_S)
        for k_i, bc in enumerate(sum_rows):
            rows = slice(bc * 128, (bc + 1) * 128)
            u_part = su[:, k_i, :7].sum(axis=1) + su[:, k_i, 9]
            z_r[0, rows] += u_part + su[:, k_i, 7]
            z_r[1, rows] += u_part + su[:, k_i, 8]
        cu = ro[:, n_sum * NSP_S:].reshape(128, n_cand, NSP_C, 8)
        for k_i, bc in enumerate(cand_rows):
            for m in range(2):
                sel = list(range(15)) + [15 + 2 * m, 16 + 2 * m]
                cand_chunks[m][bc].append(
                    cu[:, k_i, sel, :].reshape(128, -1) / PSCALE)
        pad_tot += pads[c]
    z_r -= pad_tot

    z_sums = np.zeros((2, B), dtype=np.float64)
    z_sums[:, perm_rows] = z_r

    ncc = 17 * 8 * NCORES
    cand_cos = []
    for m in range(2):
        cc = np.full((B, ncc), -1.0)
        for bc in cand_rows:
            rows = slice(bc * 128, (bc + 1) * 128)
            cc[rows] = np.concatenate(cand_chunks[m][bc], axis=1)
        cc_orig = np.full_like(cc, -1.0)
        cc_orig[perm_rows] = cc
        cand_cos.append(cc_orig)

    return _host_loss(p, queue, mask_flat, label, z_sums, cand_cos)


# revision 23
# speedup vs baseline: 1.1439x; 1.0627x over previous
"""AM-softmax + hard-negative-mining loss (partial-FC style) on 8 TRN2 cores.

Strategy (classification/tensor parallel over the queue dim Q):
  - Column dedup: the blended weight w = mask*q1 + (1-mask)*q0 equals q0
    exactly where mask == 0 (~90% of columns), so the host permutes
    columns into a shared "U" block (one matmul feeding both loss terms)
    and an "M" block (both variants computed). ~45% FLOP reduction.
  - fp8(e4m3) matmuls in DoubleRow perf mode: inputs pre-scaled by 16 on
    host and quantized; each matmul contracts K=256 (two fp8 rows per PE
    cell); psum = 256*cos in fp32. fp8 end-to-end loss error ~6e-5 rel.
  - Batch rows reordered pos-first / outlier-last so each 128-row chunk
    needs only one kind of consumer: exp+rowsum (pos chunks, feeding
    logsumexp) or top-8-per-span (outlier chunks, feeding hard-negative
    top-k). That halves elementwise work vs exp+max8 everywhere.
  - A single [128, 4096] PSUM tile is used as a ring (the tile framework
    tracks subtile dependencies) with engine-exclusive regions: two
    1536-wide regions ping-pong the ACT stream (exact exp(32cos) with
    fused row-sum accumulation, in-place psum->psum, zero-gap); two
    512-wide regions feed the DVE stream.
  - DVE work: per pos chunk, U[0:1536] uses a Schraudolph bit-trick exp
    (y = int32(A*psum + B) reinterpreted as f32; the sawtooth error
    averages out inside the 65536-term logsumexp, validated ~5e-5 rel)
    followed by a reduce over the bitcast values; outlier chunks run
    max8 straight from PSUM in 512-wide spans. The two streams are
    merged by estimated engine time so ACT (~51us) and DVE (~46us) run
    concurrently; fills (PE, ~28us) and DMA (~15us) hide underneath.
  - Latency trims: per-bc p-slice DMAs + a split 512-wide first unit
    start ACT at ~4.5us; a dummy-matmul warmup starts the PE p-state
    ramp clock at ~1us (cost-model matmuls dispatched 3us after the
    ramp start run at full clock); a dummy activation pre-loads the Exp
    table during the DMA window; one merged output DMA at the end.
  - Cross-core/term merge (logsumexp adjust at the ground-truth column,
    top-k merge, masked means) happens on host in float64.
"""
import sys

sys.path.insert(0, "/opt/trn_rl_repo")

import numpy as np
import ml_dtypes

B = 1024
Q = 65536
D = 512
MARGIN = 0.4
SCALE = 32.0
HARD_NEG = 10
NCORES = 8
BC = B // 128              # 8 batch chunks

NU = 7424                  # U (shared) columns per core; capacity 59392
NM = 832                   # M (masked) columns per core; capacity 6656
# Column chunks: the D chunk (U[0:1536]) is consumed via DVE approx-exp
# for pos chunks; A chunks via ACT exact exp. M0/M1 are the two masked
# variants. For outlier (neg) chunks everything is consumed by DVE max8
# in 512-wide spans.
D_W = 2048                 # U columns offloaded to DVE per pos chunk
A_CHUNKS = [(2048, 1536), (3584, 1280), (4864, 1280), (6144, 1280)]
NSP_S = 7                  # sum spans: a0 a1 a2 a3 m0 m1 a0x
NSP_C = 20                 # cand spans (512-wide per neg chunk)
FSCALE = 16.0              # host pre-scale on p and q before fp8 quant
PSCALE = FSCALE * FSCALE   # psum = PSCALE * cos
MMW = 512                  # output cols per DoubleRow matmul
RING = 4096                # psum ring size (fp32 elements; 8 banks)

# Schraudolph approx exp on psum values x = PSCALE*cos:
#   exp(SCALE*cos) ~ bitcast_f32(int32(EXPA * x + EXPB))
EXPA = (2.0 ** 23) * 1.4426950408889634 * (SCALE / PSCALE)
EXPB = float((127 << 23) - 486411)

QS = Q // NCORES           # generic-fallback shard size
PW = 1024                  # generic fallback tile width
NSP_G = QS // PW

TRACE = False
LAST = {}

_NC_CACHE = {}


def _build_fast(kinds):
    """kinds: per-bc tuple of (needs_sum, needs_cand)."""
    key = ("fast", kinds)
    if key in _NC_CACHE:
        return _NC_CACHE[key]
    import concourse.mybir as mybir
    import concourse.tile as tile
    from concourse import bacc

    dt = mybir.dt
    f8 = dt.float8e4
    EXP = mybir.ActivationFunctionType.Exp
    DR = mybir.MatmulPerfMode.DoubleRow
    AX = mybir.AxisListType.X
    ALU = mybir.AluOpType
    nc = bacc.Bacc(None)

    pQ = nc.dram_tensor("pQ", [128, 2, 2, B], dt.uint8, kind="ExternalInput")
    qU = nc.dram_tensor("qU", [128, 2, 2, NU], dt.uint8, kind="ExternalInput")
    qM = nc.dram_tensor("qM", [128, 2, 2, 2 * NM], dt.uint8,
                        kind="ExternalInput")
    n_sum = sum(1 for s, _ in kinds if s)
    n_cand = sum(1 for _, c in kinds if c)
    n_out = n_sum * NSP_S + n_cand * NSP_C * 8
    out = nc.dram_tensor("out", [128, n_out], dt.float32,
                         kind="ExternalOutput")
    oint = nc.dram_tensor("oint", [128, max(n_sum, 1), D_W], dt.int32,
                          kind="ExternalOutput")

    pos_bcs = [bc for bc in range(BC) if kinds[bc][0]]
    neg_bcs = [bc for bc in range(BC) if not kinds[bc][0]]
    si_row = {bc: i for i, bc in enumerate(bc for bc in range(BC)
                                           if kinds[bc][0])}
    ci_row = {bc: i for i, bc in enumerate(bc for bc in range(BC)
                                           if kinds[bc][1])}

    with tile.TileContext(nc) as tc:
        with (
            tc.tile_pool(name="const", bufs=1) as cpool,
            tc.tile_pool(name="scr", bufs=3) as spool,
            tc.tile_pool(name="ps", bufs=1, space="PSUM") as ps,
        ):
            ring = ps.tile([128, RING], dt.float32, name="ring")

            # -- warmups: start PE ramp clock + load ACT Exp table early
            wt = cpool.tile([128, 16], f8, name="wt")
            nc.vector.memset(wt[:], 0.0)
            wa = cpool.tile([128, 8], dt.float32, name="wa")
            nc.vector.memset(wa[:], 0.0)
            for i in range(12):
                nc.tensor.matmul(ring[0:1, 0:8], wt[:, 0:1], wt[:, 8:16],
                                 start=True, stop=True)
            nc.scalar.activation(wa[:], wa[:], EXP, scale=1.0)

            # DMA order = consumption order. The very first compute unit
            # is a 512-wide exp for bc0, so ship bc0's p slice and the
            # first 512 queue columns first to start ACT ~4us earlier.
            bc_order = pos_bcs + neg_bcs
            pQt = cpool.tile([128, 2, 2, B], f8, name="pQt")
            qUt = cpool.tile([128, 2, 2, NU], f8, name="qUt")

            def dma_pq(bc):
                b0 = bc * 128
                nc.sync.dma_start(pQt[:, :, :, b0:b0 + 128],
                                  pQ[:, :, :, b0:b0 + 128].bitcast(f8))

            # interleave per-bc p slices with the first queue chunks so the
            # k-th consumer unit's inputs arrive as early as possible
            dma_pq(bc_order[0])
            nc.sync.dma_start(qUt[:, :, :, 2048:2560],
                              qU[:, :, :, 2048:2560].bitcast(f8))
            dma_pq(bc_order[1])
            nc.sync.dma_start(qUt[:, :, :, 2560:3584],
                              qU[:, :, :, 2560:3584].bitcast(f8))
            for bc in bc_order[2:5]:
                dma_pq(bc)
            nc.sync.dma_start(qUt[:, :, :, 0:1024],
                              qU[:, :, :, 0:1024].bitcast(f8))
            nc.sync.dma_start(qUt[:, :, :, 1024:2048],
                              qU[:, :, :, 1024:2048].bitcast(f8))
            for bc in bc_order[5:]:
                dma_pq(bc)
            for c0, w in A_CHUNKS[1:]:
                nc.sync.dma_start(qUt[:, :, :, c0:c0 + w],
                                  qU[:, :, :, c0:c0 + w].bitcast(f8))
            qMt = cpool.tile([128, 2, 2, 2 * NM], f8, name="qMt")
            nc.sync.dma_start(qMt[:], qM[:].bitcast(f8))

            ointt = cpool.tile([128, max(n_sum, 1), D_W], dt.int32,
                               name="ointt")
            outs = cpool.tile([128, max(n_out, 1)], dt.float32, name="outs")
            sums = outs[:, :n_sum * NSP_S].rearrange(
                "p (r s) -> p r s", s=NSP_S) if n_sum else outs
            cands = outs[:, n_sum * NSP_S:].rearrange(
                "p (r s) -> p r s", s=NSP_C * 8) if n_cand else outs
            nc.vector.memset(outs[:, :max(n_sum, 1) * NSP_S], 0.0)

            # Two independent unit streams on disjoint ring regions:
            #   ACT stream: R0 [0:1536] / R1 [1536:3072], alternating
            #   DVE stream: R2a [3072:3584] / R2b [3584:4096], alternating
            # ACT stream: (bc, src, c0, w, span) exact exp + accum, pos bcs
            act_stream = []
            for k, (c0, w) in enumerate(A_CHUNKS):
                for j, bc in enumerate(pos_bcs):
                    if k == 0 and j == 0:
                        # split bc0's first unit so ACT starts right after
                        # the small lead DMAs
                        act_stream.append((bc, qUt, c0, 512, 6))
                        act_stream.append((bc, qUt, c0 + 512, w - 512, 0))
                    else:
                        act_stream.append((bc, qUt, c0, w, k))
            for bc in pos_bcs:
                act_stream.append((bc, qMt, 0, NM, 4))
            for bc in pos_bcs:
                act_stream.append((bc, qMt, NM, NM, 5))

            # DVE stream: approx-exp D spans (pos) + max8 spans (neg),
            # all 512-wide. (kind, bc, src, c0, w, span)
            dve_stream = []
            nd = D_W // 512
            for j in range(nd):
                for bc in pos_bcs:
                    dve_stream.append(("exp", bc, qUt, j * 512, 512, j))
                    if j == nd - 1:
                        dve_stream.append(("dump", bc, None, 0, 0, 0))
            cs = 0
            for c0, w in [(0, D_W)] + A_CHUNKS:
                for h0 in range(0, w, 512):
                    hw = min(512, w - h0)
                    for bc in neg_bcs:
                        dve_stream.append(
                            ("max", bc, qUt, c0 + h0, hw, cs))
                    cs += 1
            for mv in range(2):
                for h0 in range(0, NM, 512):
                    hw = min(512, NM - h0)
                    for bc in neg_bcs:
                        dve_stream.append(
                            ("max", bc, qMt, mv * NM + h0, hw, cs))
                    cs += 1
            assert cs == NSP_C

            # merge the two streams by estimated engine time so both
            # engines stay fed from the shared fill producer
            def a_cost(u):
                return u[3] * 0.83 + 330.0

            def d_cost(u):
                if u[0] == "dump":
                    return 50.0
                return u[4] * (1.1 if u[0] == "exp" else 1.05) + 150.0

            plan = []
            ta = td = 0.0
            ia = idv = 0
            # seed: the first A-units go first (their q chunk is DMA'd
            # first and PE fills are in program order)
            seed = min(4, len(act_stream))
            for u in act_stream[:seed]:
                plan.append(("A",) + u)
                ta += a_cost(u)
            ia = seed
            td = ta * 0.45
            while ia < len(act_stream) or idv < len(dve_stream):
                if idv >= len(dve_stream) or (
                        ia < len(act_stream) and
                        ta + a_cost(act_stream[ia]) <=
                        td + d_cost(dve_stream[idv])):
                    plan.append(("A",) + act_stream[ia])
                    ta += a_cost(act_stream[ia])
                    ia += 1
                else:
                    plan.append(("D",) + dve_stream[idv])
                    td += d_cost(dve_stream[idv])
                    idv += 1

            flip = {"A": 0, "D": 0}
            base_for = {("A", 0): 0, ("A", 1): 1536,
                        ("D", 0): 3072, ("D", 1): 3584}

            for item in plan:
                if item[0] == "A":
                    _, bc, src, c0, w, sj = item
                    dkind = None
                else:
                    _, dkind, bc, src, c0, w, sj = item
                    if dkind == "dump":
                        r_i = si_row[bc]
                        nc.sync.dma_start(oint[:, r_i, :], ointt[:, r_i, :])
                        continue
                eng = item[0]
                flip[eng] ^= 1
                base = base_for[(eng, flip[eng])]
                acc = ring[:, base:base + w]
                for h0 in range(0, w, MMW):
                    hw = min(MMW, w - h0)
                    for dc in range(2):
                        nc.tensor.matmul(
                            acc[:, h0:h0 + hw],
                            pQt[:, dc, :, bc * 128:(bc + 1) * 128],
                            src[:, dc, :, c0 + h0:c0 + h0 + hw],
                            start=(dc == 0),
                            stop=(dc == 1),
                            perf_mode=DR,
                        )
                if eng == "A":
                    nc.scalar.activation(
                        acc, acc, EXP, scale=SCALE / PSCALE,
                        accum_out=sums[:, si_row[bc], sj:sj + 1],
                    )
                elif dkind == "exp":
                    # schraudolph approx exp to int32 (frees the ring
                    # region); raw ints ship to DRAM, host sums the
                    # bitcast floats in fp64
                    nc.vector.tensor_scalar(
                        ointt[:, si_row[bc], c0:c0 + w], acc,
                        EXPA, EXPB, ALU.mult, ALU.add)
                else:
                    nc.vector.max(
                        out=cands[:, ci_row[bc], sj * 8:(sj + 1) * 8],
                        in_=acc)

            nc.sync.dma_start(out[:], outs[:])

    nc.compile()
    _NC_CACHE[key] = nc
    return nc


def _q_layout(rows, n_cols):
    """[k, D] fp8-bytes (k <= n_cols) -> [128, 2, 2, n_cols] uint8, zero pad.
    Element (pp, dc, i, j) = rows[j, dc*256 + i*128 + pp]."""
    out = np.zeros((128, 2, 2, n_cols), dtype=np.uint8)
    k = rows.shape[0]
    if k:
        t = np.ascontiguousarray(rows.T).reshape(2, 2, 128, k)
        out[:, :, :, :k] = t.transpose(2, 0, 1, 3)
    return np.ascontiguousarray(out)


def _fp8(x):
    return (np.asarray(x, np.float32) * FSCALE).astype(
        ml_dtypes.float8_e4m3).view(np.uint8)


# ---------------------------------------------------------------------------
# generic fallback (dense masks): f32r path, every column handled as masked
# ---------------------------------------------------------------------------

def _build_generic():
    if "gen" in _NC_CACHE:
        return _NC_CACHE["gen"]
    import concourse.mybir as mybir
    import concourse.tile as tile
    from concourse import bacc

    dt = mybir.dt
    nc = bacc.Bacc(None)
    f32r = dt.float32r
    EXP = mybir.ActivationFunctionType.Exp
    DCg = D // 128
    pT = nc.dram_tensor("pT", [DCg, 128, B], dt.float32, kind="ExternalInput")
    q0T = nc.dram_tensor("q0T", [128, DCg, QS], dt.float32, kind="ExternalInput")
    wT = nc.dram_tensor("wT", [128, DCg, QS], dt.float32, kind="ExternalInput")
    osums = nc.dram_tensor("osums", [2, BC, 128, NSP_G], dt.float32, kind="ExternalOutput")
    ocand = nc.dram_tensor("ocand", [2, BC, 128, NSP_G * 8], dt.float32, kind="ExternalOutput")

    with tile.TileContext(nc) as tc:
        with (
            tc.tile_pool(name="const", bufs=1) as cpool,
            tc.tile_pool(name="qin", bufs=4) as qpool,
            tc.tile_pool(name="accum", bufs=1) as apool,
            tc.tile_pool(name="scr", bufs=3) as spool,
            tc.tile_pool(name="ps", bufs=4, space="PSUM") as ps,
        ):
            pTr = cpool.tile([128, DCg, B], f32r, tag="pTr", name="pTr")
            for dcg in range(DCg):
                nc.sync.dma_start(pTr[:, dcg, :], pT[dcg].bitcast(f32r))

            sums = [[apool.tile([128, NSP_G], dt.float32, tag=f"s{m}_{bc}",
                                name=f"s{m}_{bc}") for bc in range(BC)]
                    for m in range(2)]
            cand = [[apool.tile([128, NSP_G * 8], dt.float32, tag=f"c{m}_{bc}",
                                name=f"c{m}_{bc}") for bc in range(BC)]
                    for m in range(2)]

            for m, src_dram in enumerate((q0T, wT)):
                for si in range(NSP_G):
                    off = si * PW
                    qt = qpool.tile([128, DCg, PW], f32r, tag="q",
                                    name=f"g{m}q{si}")
                    for dcg in range(DCg):
                        nc.sync.dma_start(
                            qt[:, dcg, :],
                            src_dram[:, dcg, off:off + PW].bitcast(f32r))
                    for bc in range(BC):
                        acc = ps.tile([128, PW], dt.float32, tag="ps",
                                      name=f"g{m}a{si}_{bc}")
                        for h0 in range(0, PW, 512):
                            for dcg in range(DCg):
                                nc.tensor.matmul(
                                    acc[:, h0:h0 + 512],
                                    pTr[:, dcg, bc * 128:(bc + 1) * 128],
                                    qt[:, dcg, h0:h0 + 512],
                                    start=(dcg == 0),
                                    stop=(dcg == DCg - 1),
                                )
                        et = spool.tile([128, PW], dt.float32, tag="et",
                                        name=f"g{m}e{si}_{bc}")
                        nc.scalar.activation(
                            et[:], acc[:], EXP, scale=SCALE,
                            accum_out=sums[m][bc][:, si:si + 1],
                        )
                        nc.vector.max(
                            out=cand[m][bc][:, si * 8:(si + 1) * 8],
                            in_=et[:])

            for m in range(2):
                for bc in range(BC):
                    nc.sync.dma_start(osums[m, bc], sums[m][bc][:])
                    nc.sync.dma_start(ocand[m, bc], cand[m][bc][:])

    nc.compile()
    _NC_CACHE["gen"] = nc
    return nc


def _layoutT(cols_2d, n_cols):
    DCg = D // 128
    out = np.zeros((128, DCg, n_cols), dtype=np.float32)
    k = cols_2d.shape[0]
    if k:
        t = np.ascontiguousarray(cols_2d.T).reshape(DCg, 128, k)
        out[:, :, :k] = t.transpose(1, 0, 2)
    return np.ascontiguousarray(out)


def _host_loss(p, queue, mask_flat, label, z_sums, cand_cos):
    """z_sums: [2, B] fp64 raw exp-sums (pads already removed);
    cand_cos: [2][B, ncand] fp64 candidate cos values."""
    pos_mask = label != -1
    n_pos = int(pos_mask.sum())
    n_neg = B - n_pos
    p64 = p.astype(np.float64)
    q64 = queue.astype(np.float64)
    m64 = mask_flat.astype(np.float64)

    loss = 0.0
    for m in range(2):
        if n_pos > 0:
            lbl = label[pos_mask]
            if m == 0:
                w_rows = q64[0, lbl, :]
            else:
                mm = m64[lbl][:, None]
                w_rows = mm * q64[1, lbl, :] + (1.0 - mm) * q64[0, lbl, :]
            gt = np.einsum("bd,bd->b", p64[pos_mask], w_rows)
            z = z_sums[m][pos_mask]
            z_adj = z - np.exp(SCALE * gt) + np.exp(SCALE * (gt - MARGIN))
            ce = np.log(z_adj) - (gt - MARGIN) * SCALE
            loss += ce.sum() / max(n_pos, 1)
        if n_neg > 0:
            co = cand_cos[m][~pos_mask]
            topk = -np.partition(-co, HARD_NEG - 1, axis=1)[:, :HARD_NEG]
            hard = np.clip(topk, 0.0, None)
            loss += hard.mean(axis=1).sum() / max(n_neg, 1)
    return np.float32(loss)


def kernel(p, queue, mask, label):
    from concourse.bass_utils import run_bass_kernel_spmd

    p = np.ascontiguousarray(np.asarray(p, dtype=np.float32))
    queue = np.asarray(queue, dtype=np.float32)
    mask_flat = np.asarray(mask, dtype=np.float32).reshape(-1)
    label = np.asarray(label).astype(np.int64).reshape(-1)

    mask_nz = mask_flat != 0.0
    idx_M = np.nonzero(mask_nz)[0]
    idx_U = np.nonzero(~mask_nz)[0]
    use_fast = len(idx_M) <= NCORES * NM and len(idx_U) <= NCORES * NU

    core_ids = list(range(NCORES))
    kw = {}
    if TRACE:
        kw = dict(trace=True, trace_cores=[0])

    if not use_fast:
        # dense/sparse-extreme masks: f32r generic path (2 matmuls/col)
        perm = np.concatenate([idx_U, idx_M])
        q0p = queue[0, perm, :]
        mcol = mask_flat[perm][:, None]
        wp = (mcol * queue[1, perm, :] + (1.0 - mcol) * queue[0, perm, :]
              ).astype(np.float32)
        pT = np.ascontiguousarray(p.T).reshape(D // 128, 128, B)
        in_maps = []
        for c in core_ids:
            sl = slice(c * QS, (c + 1) * QS)
            in_maps.append({
                "pT": pT,
                "q0T": _layoutT(q0p[sl], QS),
                "wT": _layoutT(wp[sl], QS),
            })
        nc = _build_generic()
        try:
            res = run_bass_kernel_spmd(nc, in_maps, core_ids, **kw)
        except ModuleNotFoundError:
            res = run_bass_kernel_spmd(nc, in_maps, core_ids)
        LAST["res"] = res
        z_sums = np.zeros((2, B), dtype=np.float64)
        cands = [[], []]
        for c in core_ids:
            r = res.results[c]
            z_sums += r["osums"].astype(np.float64).sum(axis=3).reshape(2, B)
            cm = r["ocand"].astype(np.float64).reshape(2, B, NSP_G * 8)
            cands[0].append(cm[0])
            cands[1].append(cm[1])
        with np.errstate(divide="ignore"):
            cand_cos = [np.log(np.concatenate(cands[0], axis=1)) / SCALE,
                        np.log(np.concatenate(cands[1], axis=1)) / SCALE]
        return _host_loss(p, queue, mask_flat, label, z_sums, cand_cos)

    # ---- fast path ----
    pos_mask_orig = label != -1
    perm_rows = np.argsort(~pos_mask_orig, kind="stable")
    p_r = p[perm_rows]
    pos_r = pos_mask_orig[perm_rows]
    kinds = tuple(
        (bool(pos_r[bc * 128:(bc + 1) * 128].any()),
         bool((~pos_r[bc * 128:(bc + 1) * 128]).any()))
        for bc in range(BC))

    q0 = queue[0]
    mcolM = mask_flat[idx_M][:, None]
    wM = (mcolM * queue[1, idx_M, :]
          + (1.0 - mcolM) * queue[0, idx_M, :]).astype(np.float32)

    p8 = _fp8(p_r)                     # [B, D] u8
    pQ = np.ascontiguousarray(
        p8.T.reshape(2, 2, 128, B).transpose(2, 0, 1, 3))
    q0_8 = _fp8(q0)
    wM_8 = _fp8(wM)

    in_maps = []
    pads = []
    for c in core_ids:
        iu = idx_U[c * NU:(c + 1) * NU]
        im = slice(c * NM, min((c + 1) * NM, len(idx_M)))
        m_rows = wM_8[im]
        m0_rows = q0_8[idx_M[im]]
        qm = np.zeros((128, 2, 2, 2 * NM), dtype=np.uint8)
        qm[:, :, :, :NM] = _q_layout(m0_rows, NM)
        qm[:, :, :, NM:] = _q_layout(m_rows, NM)
        in_maps.append({
            "pQ": pQ,
            "qU": _q_layout(q0_8[iu], NU),
            "qM": qm,
        })
        pads.append((NU - len(iu)) + (NM - m0_rows.shape[0]))

    nc = _build_fast(kinds)
    try:
        res = run_bass_kernel_spmd(nc, in_maps, core_ids, **kw)
    except ModuleNotFoundError:
        res = run_bass_kernel_spmd(nc, in_maps, core_ids)
    LAST["res"] = res

    # ---- host-side reduction (float64) ----
    n_sum = sum(1 for s, _ in kinds if s)
    n_cand = sum(1 for _, c in kinds if c)
    sum_rows = [bc for bc in range(BC) if kinds[bc][0]]
    cand_rows = [bc for bc in range(BC) if kinds[bc][1]]

    z_r = np.zeros((2, B), dtype=np.float64)
    cand_chunks = [[[] for _ in range(BC)] for _ in range(2)]
    pad_tot = 0.0
    for c in core_ids:
        r = res.results[c]
        ro = r["out"].astype(np.float64)
        su = ro[:, :n_sum * NSP_S].reshape(128, n_sum, NSP_S)
        for k_i, bc in enumerate(sum_rows):
            rows = slice(bc * 128, (bc + 1) * 128)
            dsum = np.ascontiguousarray(
                r["oint"][:, k_i, :]).view(np.float32).astype(
                np.float64).sum(axis=1)
            u_part = su[:, k_i, :4].sum(axis=1) + su[:, k_i, 6] + dsum
            z_r[0, rows] += u_part + su[:, k_i, 4]
            z_r[1, rows] += u_part + su[:, k_i, 5]
        cu = ro[:, n_sum * NSP_S:].reshape(128, n_cand, NSP_C, 8)
        for k_i, bc in enumerate(cand_rows):
            for m in range(2):
                sel = list(range(16)) + [16 + 2 * m, 17 + 2 * m]
                cand_chunks[m][bc].append(
                    cu[:, k_i, sel, :].reshape(128, -1) / PSCALE)
        pad_tot += pads[c]
    z_r -= pad_tot

    z_sums = np.zeros((2, B), dtype=np.float64)
    z_sums[:, perm_rows] = z_r

    ncc = 18 * 8 * NCORES
    cand_cos = []
    for m in range(2):
        cc = np.full((B, ncc), -1.0)
        for bc in cand_rows:
            rows = slice(bc * 128, (bc + 1) * 128)
            cc[rows] = np.concatenate(cand_chunks[m][bc], axis=1)
        cc_orig = np.full_like(cc, -1.0)
        cc_orig[perm_rows] = cc
        cand_cos.append(cc_orig)

    return _host_loss(p, queue, mask_flat, label, z_sums, cand_cos)
